# revision 1
# baseline (speedup 1.0000x reference)
"""Bass/Tile Trainium2 kernel for causal MHA with RoPE (nn_MultiHeadAttention).

Problem (hardcoded): x [2,2048,1024] fp32, w_qkv [1024,3072], w_out [1024,1024],
16 heads, head_dim 64, rope theta 10000, causal softmax, out proj.

Sharding: 8 cores = 2 batches x 4 head-groups (4 heads each), Megatron-style
QKV column-split + out-proj row-split; host sums the 4 partial outputs per
batch.

Per-core design (all-transposed layout):
  qT/kT [d,s] f32r matmuls with RoPE fused at PSUM eviction (interleaved
  pair layout -> stream_shuffle pair swap), scoresT [sk,sq] with 2-head K=64
  row packing, causal mask added in PSUM via identity-matmul of a -1e9 mask,
  exp on ACT without max subtraction (scores are O(1)), row sums l via
  ones-matmuls packed 4-per-bank, 1/l broadcast via K=1 PE matmul,
  fp16 normalized probsT, PV with 2-head col packing, fp16 out-projection,
  all pipelined per 512-wide query superblock.
"""
import sys

sys.path.insert(0, "/opt/trn_rl_repo")
import numpy as np

N_EMBD = 1024
N_HEAD = 16
D = 64
B = 2
S = 2048
THETA = 10000.0

_PROG_CACHE = {}

PAIRSWAP = []
for _i in range(16):
    PAIRSWAP += [2 * _i + 1, 2 * _i]


def _subs(W):
    out = []
    off = 0
    while off < W:
        w = min(1024, W - off)
        out.append((off, w))
        off += w
    return out


def _build_program(n_reps=1, loop_n=0, stage="full"):
    import concourse.bass as bass
    from concourse import bacc
    import concourse.mybir as mybir
    from concourse.tile import TileContext
    from concourse import library_config

    f32 = mybir.dt.float32
    f32r = mybir.dt.float32r
    f16 = mybir.dt.float16
    EXP = mybir.ActivationFunctionType.Exp

    nc = bacc.Bacc("TRN2", target_bir_lowering=False, debug=False)

    xT = nc.declare_dram_parameter("xT", [1024, 2048], f32r, isOutput=False)
    wq = nc.declare_dram_parameter("wq", [128, 2048], f32r, isOutput=False)
    wk = nc.declare_dram_parameter("wk", [128, 2048], f32r, isOutput=False)
    wv = nc.declare_dram_parameter("wv", [128, 2048], f32r, isOutput=False)
    wo = nc.declare_dram_parameter("wo", [2, 128, 1024], f16, isOutput=False)
    cosT = nc.declare_dram_parameter("cosT", [128, 2048], f32, isOutput=False)
    sinT = nc.declare_dram_parameter("sinT", [128, 2048], f32, isOutput=False)
    msk = nc.declare_dram_parameter("msk", [128, 4, 512], f32r, isOutput=False)
    iden = nc.declare_dram_parameter("iden", [128, 128], f32r, isOutput=False)
    y = nc.declare_dram_parameter("y", [2048, 1024], f16, isOutput=True)

    MM = nc.tensor.matmul

    with nc.allow_low_precision(reason="fp16 probs/out path is intentional"), \
         TileContext(nc) as tc:
      from contextlib import ExitStack as _ES
      for _rep in range(n_reps):
        if _rep:
            tc.strict_bb_all_engine_barrier()
        with _ES() as _loopctx:
          if loop_n:
            _loopctx.enter_context(tc.For_i(0, loop_n, 1))
          with (
              tc.tile_pool(name="persist", bufs=1) as pp,
              tc.tile_pool(name="small", bufs=3) as sp,
          ):
              nc.gpsimd.load_library(library_config.standard)

              ones_col = pp.tile([128, 1], f16, tag="ones_col")
              ones_all = pp.tile([128, 128], f16, tag="ones_all")
              nc.gpsimd.memset(ones_col[:], 1.0)
              nc.gpsimd.memset(ones_all[:], 1.0)

              qsb = [pp.tile([128, 2048], f32r, tag=f"q{h}", name=f"q{h}") for h in range(2)]
              ksb = [pp.tile([128, 2048], f32r, tag=f"k{h}", name=f"k{h}") for h in range(2)]
              v_sb = pp.tile([128, 16, 256], f16, tag="v")
              outT = [pp.tile([128, 2048], f16, tag=f"o{h}", name=f"oT{h}") for h in range(2)]
              wo_sb = pp.tile([128, 2, 1024], f16, tag="wo")
              R_sb = [pp.tile([128, 2048], f16, tag=f"R{p}", name=f"R{p}") for p in range(2)]
              msk_sb = pp.tile([128, 4, 512], f32r, tag="msk")
              id_sb = pp.tile([128, 128], f32r, tag="iden")

              # ---------------- Phase 1: QKV + RoPE ----------------
              with (
                  tc.tile_pool(name="qkvp", bufs=1) as qp,
                  tc.tile_pool(name="rope", bufs=3) as rp,
                  tc.tile_pool(name="ps_qkv", bufs=1, space="PSUM") as ps1,
              ):
                  x_sb = qp.tile([128, 8, 2048], f32r, tag="x")
                  wq_sb = qp.tile([128, 2048], f32r, tag="wq")
                  wk_sb = qp.tile([128, 2048], f32r, tag="wk")
                  wv_sb = qp.tile([128, 2048], f32r, tag="wv")
                  cos_sb = qp.tile([128, 2048], f32, tag="cos")
                  sin_sb = qp.tile([128, 2048], f32, tag="sin")

                  nc.sync.dma_start(out=wq_sb[:], in_=wq[:])
                  nc.sync.dma_start(out=wk_sb[:], in_=wk[:])
                  for et in range(8):
                      nc.sync.dma_start(out=x_sb[:, et, :], in_=xT[et * 128:(et + 1) * 128, :])
                  nc.sync.dma_start(out=wv_sb[:], in_=wv[:])
                  nc.sync.dma_start(out=cos_sb[:], in_=cosT[:])
                  nc.sync.dma_start(out=sin_sb[:], in_=sinT[:])
                  nc.sync.dma_start(out=msk_sb[:], in_=msk[:])
                  nc.sync.dma_start(out=id_sb[:], in_=iden[:])
                  for hp in range(2):
                      nc.sync.dma_start(out=wo_sb[:, hp, :], in_=wo[hp])

                  if stage == "dma":
                      _skip_qkv = True
                  else:
                      _skip_qkv = False
                  # q and k in transposed layout, RoPE fused at eviction
                  # (hp outer so head-pair 0 is ready as early as possible)
                  for hp in range(2 if not _skip_qkv else 0):
                      for (wsb, dst, tg) in ((wq_sb, qsb, "q"), (wk_sb, ksb, "k")):
                          for c4 in range(4):
                              sl = slice(c4 * 512, (c4 + 1) * 512)
                              pq = ps1.tile([128, 512], f32, tag=f"ps_{tg}", bufs=3)
                              for et in range(8):
                                  MM(pq[:], wsb[:, (et * 2 + hp) * 128:(et * 2 + hp + 1) * 128],
                                     x_sb[:, et, sl], start=(et == 0), stop=(et == 7))
                              t1 = rp.tile([128, 512], f32, tag="t1")
                              nc.vector.tensor_mul(t1[:], pq[:], cos_sb[:, sl])
                              rot = rp.tile([128, 512], f32, tag="rot")
                              nc.vector.stream_shuffle(rot[:], pq[:], PAIRSWAP)
                              t2 = rp.tile([128, 512], f32, tag="t2")
                              nc.gpsimd.tensor_mul(t2[:], rot[:], sin_sb[:, sl])
                              nc.gpsimd.tensor_add(dst[hp][:, sl], t1[:], t2[:])

                  # v in natural layout [s, d_local]
                  for st in range(16 if not _skip_qkv else 0):
                      pv = ps1.tile([128, 256], f32, tag="ps_v", bufs=2)
                      for et in range(8):
                          MM(pv[:], x_sb[:, et, st * 128:(st + 1) * 128],
                             wv_sb[:, et * 256:(et + 1) * 256], start=(et == 0), stop=(et == 7))
                      nc.scalar.copy(v_sb[:, st, :], pv[:])

              # -------- Phase 2+3: attention + outproj, pipelined per superblock ----
              if stage in ("dma", "qkv"):
                  nc.finalize  # no-op reference
              with (
                  tc.tile_pool(name="probs", bufs=1) as probp,
                  tc.tile_pool(name="ps_att", bufs=1, space="PSUM") as ps2,
              ):
                  for hp in range(2 if stage not in ("dma", "qkv") else 0):
                      probs = {}
                      l_ps = [ps2.tile([128, 512], f32, tag=f"l{p}", name=f"l{p}")
                              for p in range(2)]
                      for j in range(16):
                          sq0 = (j // 4) * 512
                          W = 2048 - sq0
                          jj = slice(j * 128, (j + 1) * 128)
                          for par in range(2):
                              probs[(j, par)] = probp.tile(
                                  [128, W], f16, tag=f"p{j}_{par}", name=f"pr{j}_{par}", bufs=2 if j <= 5 else 1)
                          for (off, w) in _subs(W):
                              psT = [ps2.tile([128, 1024], f32, tag=f"sT{p}", name=f"sT{p}")
                                     for p in range(2)]
                              for half in range(w // 512):
                                  qsl = slice(sq0 + off + half * 512,
                                              sq0 + off + (half + 1) * 512)
                                  diag = (off == 0 and half == 0)
                                  m = j % 4
                                  for par in range(2):
                                      pb = slice(64 * par, 64 * par + 64)
                                      osl = slice(half * 512, (half + 1) * 512)
                                      if diag:
                                          # causal mask add into PSUM: Id.T @ mask_j
                                          # (mask only nonzero in first (m+1)*128
                                          # cols; scores fully masked below m*128)
                                          wm = max(256, (m + 1) * 128)
                                          MM(psT[par][:, 0:wm], id_sb[:],
                                             msk_sb[:, m, 0:wm],
                                             start=True, stop=False,
                                             skip_group_check=True)
                                          ws = max(256, 512 - m * 128)
                                          MM(psT[par][:, 512 - ws:512],
                                             ksb[hp][pb, jj],
                                             qsb[hp][pb, sq0 + 512 - ws:sq0 + 512],
                                             start=False, stop=True,
                                             skip_group_check=True)
                                      else:
                                          MM(psT[par][:, osl],
                                             ksb[hp][pb, jj], qsb[hp][pb, qsl],
                                             start=True, stop=True,
                                             skip_group_check=True)
                              for par in range(2):
                                  nc.scalar.activation(
                                      probs[(j, par)][:, off:off + w], psT[par][:, :w], EXP)
                          # l accumulation (row sums via ones matmuls, 4 rows per bank)
                          for par in range(2):
                              for I in range(j // 4, 4):
                                  psl = slice(I * 512 - sq0, I * 512 - sq0 + 512)
                                  MM(l_ps[par][32 * I:32 * I + 1, :], ones_col[:],
                                     probs[(j, par)][:, psl],
                                     start=(j == 0), stop=(j == 4 * I + 3),
                                     skip_group_check=True,
                                     tile_position=(0, 32 * I))

                          if j % 4 == 3:
                              I = j // 4
                              Rsl = slice(I * 512, (I + 1) * 512)
                              # superblock I complete: 1/l, broadcast, normalize, PV
                              for par in range(2):
                                  rI = sp.tile([1, 512], f16, tag="r")
                                  nc.vector.reciprocal(
                                      rI[:], l_ps[par][32 * I:32 * I + 1, :])
                                  Rps = ps2.tile([128, 512], f32, tag="misc_ps", name="Rps")
                                  MM(Rps[:], ones_all[0:1, :], rI[:])
                                  if par == 0:
                                      nc.scalar.copy(R_sb[par][:, Rsl], Rps[:])
                                  else:
                                      nc.vector.tensor_copy(R_sb[par][:, Rsl], Rps[:])
                              for par in range(2):
                                  for j2 in range(4 * I + 4):
                                      sq2 = (j2 // 4) * 512
                                      psl = slice(I * 512 - sq2, I * 512 - sq2 + 512)
                                      eng = nc.gpsimd if (j2 % 4 == 1) else nc.vector
                                      eng.tensor_mul(probs[(j2, par)][:, psl],
                                                     probs[(j2, par)][:, psl],
                                                     R_sb[par][:, Rsl])
                              pvp = ps2.tile([128, 512], f32, tag="pv_ps")
                              for j2 in range(4 * I + 4):
                                  sq2 = (j2 // 4) * 512
                                  psl = slice(I * 512 - sq2, I * 512 - sq2 + 512)
                                  for par in range(2):
                                      MM(pvp[64 * par:64 * par + 64, :],
                                         v_sb[:, j2,
                                              (2 * hp + par) * 64:(2 * hp + par + 1) * 64],
                                         probs[(j2, par)][:, psl],
                                         start=(j2 == 0), stop=(j2 == 4 * I + 3),
                                         skip_group_check=True)
                              nc.vector.tensor_copy(outT[hp][:, Rsl], pvp[:])

                              if hp == 1 and stage == "full":
                                  # out projection for this superblock's 4 row tiles
                                  for t in range(4 * I, 4 * I + 4):
                                      for n in range(2):
                                          py = ps2.tile([128, 512], f32, tag="misc_ps",
                                                        name="py")
                                          for h2 in range(2):
                                              MM(py[:], outT[h2][:, t * 128:(t + 1) * 128],
                                                 wo_sb[:, h2, n * 512:(n + 1) * 512],
                                                 start=(h2 == 0), stop=(h2 == 1))
                                          ysb = sp.tile([128, 512], f16, tag="y_sb")
                                          nc.vector.tensor_copy(ysb[:], py[:])
                                          nc.sync.dma_start(
                                              out=y[t * 128:(t + 1) * 128,
                                                    n * 512:(n + 1) * 512],
                                              in_=ysb[:])

    nc.finalize()
    return nc


def _host_inputs(x, w_qkv, w_out, core):
    b = core // 4
    g = core % 4
    heads = [4 * g + i for i in range(4)]

    perm = np.empty(64, np.int64)
    for i in range(32):
        perm[2 * i] = i
        perm[2 * i + 1] = i + 32

    xT = np.ascontiguousarray(x[b].T).astype(np.float32)

    def build_wqk(Wblk, scale):
        out = np.empty((128, 2048), np.float32)
        for et in range(8):
            for hp in range(2):
                cols = np.empty((128, 128), np.float32)
                for par in range(2):
                    h = heads[2 * hp + par]
                    cols[:, par * 64:(par + 1) * 64] = (
                        Wblk[et * 128:(et + 1) * 128, h * 64 + perm])
                out[:, (et * 2 + hp) * 128:(et * 2 + hp + 1) * 128] = cols
        return (out * scale).astype(np.float32)

    wq = build_wqk(w_qkv[:, 0:1024], 0.125)
    wk = build_wqk(w_qkv[:, 1024:2048], 1.0)

    wv = np.empty((128, 2048), np.float32)
    for et in range(8):
        for hl in range(4):
            wv[:, et * 256 + hl * 64: et * 256 + (hl + 1) * 64] = (
                w_qkv[et * 128:(et + 1) * 128,
                      2048 + heads[hl] * 64: 2048 + (heads[hl] + 1) * 64])

    wo = np.empty((2, 128, 1024), np.float16)
    for hp in range(2):
        for par in range(2):
            h = heads[2 * hp + par]
            wo[hp, par * 64:(par + 1) * 64, :] = (
                w_out[h * 64:(h + 1) * 64, :].astype(np.float16))

    invf = 1.0 / (THETA ** (np.arange(0, D, 2, dtype=np.float64) / D))
    ang = np.arange(S, dtype=np.float64)[:, None] * invf[None, :]
    cosv = np.cos(ang).astype(np.float32)
    sinv = np.sin(ang).astype(np.float32)
    cosT = np.empty((128, 2048), np.float32)
    sinT = np.empty((128, 2048), np.float32)
    for r in range(64):
        i = r // 2
        sgn = -1.0 if (r % 2 == 0) else 1.0
        cosT[r, :] = cosv[:, i]
        cosT[r + 64, :] = cosv[:, i]
        sinT[r, :] = sgn * sinv[:, i]
        sinT[r + 64, :] = sgn * sinv[:, i]

    # causal masks [128, 4, 512]: msk[p, m, c] = 0 if c >= m*128+p else -1e9
    p_ = np.arange(128)[:, None, None]
    m_ = np.arange(4)[None, :, None]
    c_ = np.arange(512)[None, None, :]
    mskh = np.where(c_ >= m_ * 128 + p_, 0.0, -1e9).astype(np.float32)
    iden = np.eye(128, dtype=np.float32)

    return {
        "xT": xT, "wq": wq, "wk": wk, "wv": wv, "wo": wo,
        "cosT": cosT, "sinT": sinT, "msk": mskh, "iden": iden,
    }


def kernel(x, w_qkv, w_out, trace=False):
    from concourse.bass_utils import run_bass_kernel_spmd

    x = np.asarray(x, np.float32)
    w_qkv = np.asarray(w_qkv, np.float32)
    w_out = np.asarray(w_out, np.float32)

    if "nc" not in _PROG_CACHE:
        _PROG_CACHE["nc"] = _build_program()
    nc = _PROG_CACHE["nc"]

    in_maps = [_host_inputs(x, w_qkv, w_out, c) for c in range(8)]
    res = run_bass_kernel_spmd(nc, in_maps, list(range(8)), trace=trace)
    _PROG_CACHE["last_result"] = res

    y0 = sum(res.results[c]["y"].astype(np.float64) for c in range(4))
    y1 = sum(res.results[c]["y"].astype(np.float64) for c in range(4, 8))
    return np.stack([y0, y1]).astype(np.float32)



# revision 19
# speedup vs baseline: 1.4038x; 1.4038x over previous
"""Bass/Tile Trainium2 kernel for causal MHA with RoPE (nn_MultiHeadAttention).

Problem (hardcoded): x [2,2048,1024] fp32, w_qkv [1024,3072], w_out [1024,1024],
16 heads, head_dim 64, rope theta 10000, causal softmax, out proj.

Sharding: 8 cores = 2 batches x 4 head-groups (4 heads each), Megatron-style
QKV column-split + out-proj row-split; host sums the partial outputs per batch
(2 head-pair partials per core x 4 cores).

v2 design (vs v1 baseline at 375us):
  - f16 inputs (x, w) halve DMA; all matmuls f16 except fp32 PSUM.
  - trapezoid block-causal at 128-key granularity (query range [128j, 2048)
    per key tile j) with partial-width l/PV matmuls -> 15% less exp/scores.
  - probs stay UNNORMALIZED (exp bias -3 for headroom); l row-sums via
    ones-matmuls into per-par l_ps banks; 1/l via reciprocal_approx_fast
    once per (hp, par); broadcast via K=1 matmuls; normalization applied
    once to outT (2 muls/hp) instead of 160 probs-muls.
  - per-head-pair y partials (no h2 accumulation) so hp0 out-projection
    overlaps hp1 attention; host sums.
  - PSUM: 8 banks exactly: "big" [128,2,512] bufs=2 (4) shared by qk pq and
    scoresT psT; l_ps 2x[128,512] (2); "acc" [128,512] bufs=2 (2) shared by
    v / pvp / Rps / py.
  - program order interleaves v matmuls into hp0's j-loop and hp0's
    out-projection into hp1's j-loop to keep PE dense (HAM warm).
"""
import sys

sys.path.insert(0, "/opt/trn_rl_repo")
import numpy as np

N_EMBD = 1024
N_HEAD = 16
D = 64
B = 2
S = 2048
THETA = 10000.0

_PROG_CACHE = {}

PAIRSWAP = []
for _i in range(16):
    PAIRSWAP += [2 * _i + 1, 2 * _i]


def _build_program():
    import concourse.bass as bass
    from concourse import bacc
    import concourse.mybir as mybir
    from concourse.tile import TileContext
    from concourse import library_config

    f32 = mybir.dt.float32
    f32r = mybir.dt.float32r
    f16 = mybir.dt.float16
    EXP = mybir.ActivationFunctionType.Exp

    nc = bacc.Bacc("TRN2", target_bir_lowering=False, debug=False)

    xT = nc.declare_dram_parameter("xT", [1024, 2048], f16, isOutput=False)
    wq = nc.declare_dram_parameter("wq", [128, 2048], f16, isOutput=False)
    wk = nc.declare_dram_parameter("wk", [128, 2048], f16, isOutput=False)
    wv = nc.declare_dram_parameter("wv", [128, 2048], f16, isOutput=False)
    wo = nc.declare_dram_parameter("wo", [2, 128, 1024], f16, isOutput=False)
    cosT = nc.declare_dram_parameter("cosT", [128, 2048], f32, isOutput=False)
    sinT = nc.declare_dram_parameter("sinT", [128, 2048], f32, isOutput=False)
    msk = nc.declare_dram_parameter("msk", [128, 128], f16, isOutput=False)
    iden = nc.declare_dram_parameter("iden", [128, 128], f16, isOutput=False)
    onesr = nc.declare_dram_parameter("onesr", [1, 128], f32r, isOutput=False)
    y = nc.declare_dram_parameter("y", [2, 2048, 1024], f16, isOutput=True)

    MM = nc.tensor.matmul

    with nc.allow_low_precision(reason="f16 data path is intentional"), \
         TileContext(nc) as tc:
      with (
          tc.tile_pool(name="persist", bufs=1) as pp,
          tc.tile_pool(name="small", bufs=3) as sp,
          tc.tile_pool(name="psum", bufs=1, space="PSUM") as ps,
      ):
        nc.gpsimd.load_library(library_config.standard)

        ones_col = pp.tile([128, 1], f16, tag="ones_col")
        ones_row = pp.tile([1, 128], f32r, tag="ones_row")
        bias_sb = pp.tile([128, 1], f32, tag="bias")
        nc.gpsimd.memset(ones_col[:], 1.0)
        nc.sync.dma_start(out=ones_row[:], in_=onesr[:])
        nc.gpsimd.memset(bias_sb[:], -3.0)

        x_sb = pp.tile([128, 8, 2048], f16, tag="x")
        wv_sb = pp.tile([128, 2048], f16, tag="wv")
        msk_sb = pp.tile([128, 128], f16, tag="msk")
        id_sb = pp.tile([128, 128], f16, tag="iden")
        wo_sb = pp.tile([128, 2, 1024], f16, tag="wo")

        qsb = [pp.tile([128, 2048], f16, tag=f"q{h}", name=f"q{h}") for h in range(2)]
        ksb = [pp.tile([128, 2048], f16, tag=f"k{h}", name=f"k{h}") for h in range(2)]
        v_sb = pp.tile([128, 16, 256], f16, tag="v")
        outT = [pp.tile([128, 2048], f16, tag=f"o{h}", name=f"oT{h}") for h in range(2)]
        outN = [pp.tile([128, 2048], f16, tag=f"on{h}", name=f"oN{h}") for h in range(2)]
        rinv = [pp.tile([128, 512], f32, tag=f"ri{p}", name=f"ri{p}") for p in range(2)]
        rinv_r = [pp.tile([128, 512], f32r, tag=f"rr{p}", name=f"rr{p}")
                  for p in range(2)]
        Rcomb = pp.tile([128, 2048], f16, tag="Rc")

        # ---------------- DMA loads ----------------
        for et in range(8):
            nc.sync.dma_start(out=x_sb[:, et, :], in_=xT[et * 128:(et + 1) * 128, :])
        nc.sync.dma_start(out=wv_sb[:], in_=wv[:])
        nc.sync.dma_start(out=msk_sb[:], in_=msk[:])
        nc.sync.dma_start(out=id_sb[:], in_=iden[:])
        for hp in range(2):
            nc.sync.dma_start(out=wo_sb[:, hp, :], in_=wo[hp])

        # ---------------- QKV with fused RoPE (transposed q/k) ----------
        # rope-phase-only tiles in a scoped pool so probs can reuse the SBUF
        with tc.tile_pool(name="ropep", bufs=1) as rpp:
            wq_sb = rpp.tile([128, 2048], f16, tag="wq")
            wk_sb = rpp.tile([128, 2048], f16, tag="wk")
            cos_sb = rpp.tile([128, 2048], f32, tag="cos")
            sin_sb = rpp.tile([128, 2048], f32, tag="sin")
            nc.sync.dma_start(out=wq_sb[:], in_=wq[:])
            nc.sync.dma_start(out=wk_sb[:], in_=wk[:])
            nc.sync.dma_start(out=cos_sb[:], in_=cosT[:])
            nc.sync.dma_start(out=sin_sb[:], in_=sinT[:])

            # pq shares the "big" [128,2,512] psum tag with psT below.
            def qk_rope(hp, wsb, dst):
                for c2 in range(2):
                    big = ps.tile([128, 2, 512], f32, tag="big", name="pq",
                                  bufs=2)
                    for half in range(2):
                        c4 = c2 * 2 + half
                        sl = slice(c4 * 512, (c4 + 1) * 512)
                        pq = big[:, half, :]
                        for et in range(8):
                            MM(pq,
                               wsb[:, (et * 2 + hp) * 128:(et * 2 + hp + 1) * 128],
                               x_sb[:, et, sl], start=(et == 0), stop=(et == 7))
                        t1 = sp.tile([128, 512], f32, tag="t1")
                        nc.vector.tensor_mul(t1[:], pq, cos_sb[:, sl])
                        rot = sp.tile([128, 512], f32, tag="rot")
                        nc.vector.stream_shuffle(rot[:], pq, PAIRSWAP)
                        t2 = sp.tile([128, 512], f32, tag="t2")
                        nc.gpsimd.tensor_mul(t2[:], rot[:], sin_sb[:, sl])
                        nc.vector.tensor_add(dst[:, sl], t1[:], t2[:])

            for hp in range(2):
                qk_rope(hp, wq_sb, qsb[hp])
                qk_rope(hp, wk_sb, ksb[hp])

        def v_mm(st):
            # v natural layout [seq-tile, 4 heads x 64]
            pv = ps.tile([128, 512], f32, tag="acc", name="pv", bufs=2)
            for et in range(8):
                MM(pv[:, 0:256], x_sb[:, et, st * 128:(st + 1) * 128],
                   wv_sb[:, et * 256:(et + 1) * 256], start=(et == 0), stop=(et == 7))
            nc.vector.tensor_copy(v_sb[:, st, :], pv[:, 0:256])

        # ---------------- attention per head-pair ----------------
        l_ps = [ps.tile([128, 512], f32, tag=f"l{p}", name=f"l{p}")
                for p in range(2)]

        def attention(hp, filler, prp):
            """filler(j) emits interleaved independent PE work."""
            probs = {}
            for j in range(16):
                qa = 128 * j          # first query for this key tile
                W = 2048 - qa
                I0 = j // 4           # first (partial) superblock
                w0 = 512 - 128 * (j % 4)   # width of partial first chunk
                probs[j] = prp.tile([128, 2, W], f16, tag=f"p{j}", name=f"pr{j}",
                                    bufs=1)
                # --- scoresT chunks (chunk c covers superblock I0+c) ---
                nch = 4 - I0
                for c in range(nch):
                    qs = qa if c == 0 else 512 * (I0 + c)
                    w = w0 if c == 0 else 512
                    off = qs - qa
                    psT = ps.tile([128, 2, 512], f32, tag="big", name="psT",
                                  bufs=2)
                    for par in range(2):
                        pb = slice(64 * par, 64 * par + 64)
                        if c == 0:
                            MM(psT[:, par, 0:128], id_sb[:], msk_sb[:],
                               start=True, stop=False, skip_group_check=True)
                            MM(psT[:, par, 0:w],
                               ksb[hp][pb, j * 128:(j + 1) * 128],
                               qsb[hp][pb, qs:qs + w],
                               start=False, stop=True, skip_group_check=True)
                        else:
                            MM(psT[:, par, 0:w],
                               ksb[hp][pb, j * 128:(j + 1) * 128],
                               qsb[hp][pb, qs:qs + w],
                               start=True, stop=True, skip_group_check=True)
                    nc.scalar.activation(probs[j][:, :, off:off + w],
                                         psT[:, :, 0:w], EXP, bias=bias_sb[:])
                # --- l row-sum accumulation (per superblock I) ---
                for par in range(2):
                    for I in range(I0, 4):
                        if I == I0:
                            psl = slice(0, w0)
                            osl = slice(128 * (j % 4), 512)
                        else:
                            psl = slice(512 * I - qa, 512 * I - qa + 512)
                            osl = slice(0, 512)
                        MM(l_ps[par][32 * I:32 * I + 1, osl], ones_col[:],
                           probs[j][:, par, psl],
                           start=(j == 0), stop=(j == 4 * I + 3),
                           skip_group_check=True,
                           tile_position=(0, 32 * I))
                if filler is not None:
                    filler(j)
                # --- PV burst when superblock I = j//4 completes ---
                if j % 4 == 3:
                    I = j // 4
                    pvp = ps.tile([128, 512], f32, tag="acc", name="pvp",
                                  bufs=2)
                    for j2 in range(4 * I + 4):
                        qa2 = 128 * j2
                        if j2 // 4 == I:
                            psl = slice(0, 512 - 128 * (j2 % 4))
                            osl = slice(128 * (j2 % 4), 512)
                        else:
                            psl = slice(512 * I - qa2, 512 * I - qa2 + 512)
                            osl = slice(0, 512)
                        for par in range(2):
                            MM(pvp[64 * par:64 * par + 64, osl],
                               v_sb[:, j2, (2 * hp + par) * 64:(2 * hp + par + 1) * 64],
                               probs[j2][:, par, psl],
                               start=(j2 == 0), stop=(j2 == 4 * I + 3),
                               skip_group_check=True)
                    nc.vector.tensor_copy(outT[hp][:, 512 * I:512 * (I + 1)],
                                          pvp[:])

            # --- hp tail: 1/l, broadcast, normalize outT ---
            # All broadcast matmuls run from partition 0 (K=1, M=128): DMA
            # the needed rinv rows (32I) down to a partition-0 scratch row.
            rloc = {}
            for par in range(2):
                nc.vector.reciprocal_approx_fast(rinv[par][:], l_ps[par][:])
                nc.vector.tensor_copy(rinv_r[par][:], rinv[par][:])
                rloc[par] = sp.tile([1, 4, 512], f32r, tag=f"rl_{par}",
                                    name=f"rl_{par}", bufs=1)
                for I in range(4):
                    nc.sync.dma_start(out=rloc[par][0:1, I, :],
                                      in_=rinv_r[par][32 * I:32 * I + 1, :])
            for I in range(4):
                for par in range(2):
                    Rps = ps.tile([128, 512], f32, tag="acc", name="Rps",
                                  bufs=2)
                    MM(Rps[:], ones_row[:], rloc[par][0:1, I, :],
                       start=True, stop=True, skip_group_check=True)
                    nc.vector.tensor_copy(
                        Rcomb[64 * par:64 * par + 64, 512 * I:512 * (I + 1)],
                        Rps[64 * par:64 * par + 64, :])
            nc.vector.tensor_mul(outN[hp][:], outT[hp][:], Rcomb[:])

        def outproj(hp, t, n, evict_eng):
            py = ps.tile([128, 512], f32, tag="acc", name="py", bufs=2)
            MM(py[:], outN[hp][:, t * 128:(t + 1) * 128],
               wo_sb[:, hp, n * 512:(n + 1) * 512], start=True, stop=True)
            ysb = sp.tile([128, 512], f16, tag="y_sb")
            if evict_eng == "scalar":
                nc.scalar.copy(ysb[:], py[:])
            else:
                nc.vector.tensor_copy(ysb[:], py[:])
            nc.sync.dma_start(
                out=y[hp][t * 128:(t + 1) * 128, n * 512:(n + 1) * 512],
                in_=ysb[:])

        with tc.tile_pool(name="probsp", bufs=1) as prp:
            # hp0 attention with v matmuls interleaved
            attention(0, lambda j: v_mm(j), prp)
            # hp1 attention with hp0 out-projection interleaved
            def hp0_proj(j):
                for n in range(2):
                    outproj(0, j, n, "vector")
            attention(1, hp0_proj, prp)
            # tail: hp1 out-projection (split evictions across engines)
            for t in range(16):
                for n in range(2):
                    outproj(1, t, n, "scalar" if n else "vector")

    nc.finalize()
    return nc


def _host_inputs(x, w_qkv, w_out, core):
    b = core // 4
    g = core % 4
    heads = [4 * g + i for i in range(4)]

    perm = np.empty(64, np.int64)
    for i in range(32):
        perm[2 * i] = i
        perm[2 * i + 1] = i + 32

    xT = np.ascontiguousarray(x[b].T).astype(np.float16)

    def build_wqk(Wblk, scale):
        out = np.empty((128, 2048), np.float32)
        for et in range(8):
            for hp in range(2):
                cols = np.empty((128, 128), np.float32)
                for par in range(2):
                    h = heads[2 * hp + par]
                    cols[:, par * 64:(par + 1) * 64] = (
                        Wblk[et * 128:(et + 1) * 128, h * 64 + perm])
                out[:, (et * 2 + hp) * 128:(et * 2 + hp + 1) * 128] = cols
        return (out * scale).astype(np.float16)

    wq = build_wqk(w_qkv[:, 0:1024], 0.125)
    wk = build_wqk(w_qkv[:, 1024:2048], 1.0)

    wv = np.empty((128, 2048), np.float32)
    for et in range(8):
        for hl in range(4):
            wv[:, et * 256 + hl * 64: et * 256 + (hl + 1) * 64] = (
                w_qkv[et * 128:(et + 1) * 128,
                      2048 + heads[hl] * 64: 2048 + (heads[hl] + 1) * 64])
    wv = wv.astype(np.float16)

    wo = np.empty((2, 128, 1024), np.float16)
    for hp in range(2):
        for par in range(2):
            h = heads[2 * hp + par]
            wo[hp, par * 64:(par + 1) * 64, :] = (
                w_out[h * 64:(h + 1) * 64, :].astype(np.float16))

    invf = 1.0 / (THETA ** (np.arange(0, D, 2, dtype=np.float64) / D))
    ang = np.arange(S, dtype=np.float64)[:, None] * invf[None, :]
    cosv = np.cos(ang).astype(np.float32)
    sinv = np.sin(ang).astype(np.float32)
    cosT = np.empty((128, 2048), np.float32)
    sinT = np.empty((128, 2048), np.float32)
    for r in range(64):
        i = r // 2
        sgn = -1.0 if (r % 2 == 0) else 1.0
        cosT[r, :] = cosv[:, i]
        cosT[r + 64, :] = cosv[:, i]
        sinT[r, :] = sgn * sinv[:, i]
        sinT[r + 64, :] = sgn * sinv[:, i]

    # causal mask for the diagonal 128x128 block: col c masked for c < p
    p_ = np.arange(128)[:, None]
    c_ = np.arange(128)[None, :]
    mskh = np.where(c_ >= p_, 0.0, -30000.0).astype(np.float16)
    iden = np.eye(128, dtype=np.float16)

    return {
        "xT": xT, "wq": wq, "wk": wk, "wv": wv, "wo": wo,
        "cosT": cosT, "sinT": sinT, "msk": mskh, "iden": iden,
        "onesr": np.ones((1, 128), np.float32),
    }


def kernel(x, w_qkv, w_out, trace=False):
    from concourse.bass_utils import run_bass_kernel_spmd

    x = np.asarray(x, np.float32)
    w_qkv = np.asarray(w_qkv, np.float32)
    w_out = np.asarray(w_out, np.float32)

    if "nc" not in _PROG_CACHE:
        _PROG_CACHE["nc"] = _build_program()
    nc = _PROG_CACHE["nc"]

    in_maps = [_host_inputs(x, w_qkv, w_out, c) for c in range(8)]
    res = run_bass_kernel_spmd(nc, in_maps, list(range(8)), trace=trace)
    _PROG_CACHE["last_result"] = res

    # y[core] is [2, 2048, 1024] (per-head-pair partials); sum partials
    y0 = sum(res.results[c]["y"].astype(np.float64).sum(axis=0) for c in range(4))
    y1 = sum(res.results[c]["y"].astype(np.float64).sum(axis=0) for c in range(4, 8))
    return np.stack([y0, y1]).astype(np.float32)
